# revision 5
# baseline (speedup 1.0000x reference)
"""GCN (2-layer + mean-pool + FC + log_softmax) on 8 Trainium2 NeuronCores.

Strategy (all cores run one SPMD program; all per-core variation is in data):
  P0: every core computes xw1 = x @ W1 for ALL nodes (replicated compute,
      avoids any mid-kernel feature exchange), stores row-major f32r to HBM.
  P1: edges sharded by dst node range (12.5k nodes/core). Per 256-dst block:
      bulk-gather xw1[src] rows (dma_gather, int16 idx, 4 src-chunks of 25k),
      build selection matrix S[e, dst_local] = (iota==dl)*norm on DVE, and
      aggregate via PE matmul psum[f1, dst] += msg^T @ S (fp32r, N=256).
      Fused bias+relu (per-partition bias) gives h1^T; then xw2 = h1 @ W2
      per block, scattered to a per-core xw2 shard in HBM.
  P3: edges sharded by src node range (gather is core-local!). Layer-2
      aggregation is fused directly into graph pooling: each edge adds
      xw2[src] * norm/cnt(graph(dst)) into pool^T[f2, graph] via the same
      S-matmul trick over 4 global 256-graph windows. b2 is folded into the
      per-graph fc bias on host.
  P4: AllReduce the [64,1024] pooled partials, tiny FC + log_softmax.

kernel(**inputs) -> np.ndarray [1000, 4] float32.
"""
import sys
import numpy as np

if "/opt/trn_rl_repo" not in sys.path:
    sys.path.insert(0, "/opt/trn_rl_repo")

from concourse import bass, bacc, mybir, tile, library_config  # noqa: E402
from concourse.bass_utils import run_bass_kernel_spmd  # noqa: E402

F32 = mybir.dt.float32
I16 = mybir.dt.int16

FULL_CFG = dict(
    N=100000, E=1600000, G=1000, F0=256, F1=128, F2=64, C=4,
    NCORES=8,
    DSTW=256,      # dst-block width (layer-1 aggregation window)
    CH=25000,      # src chunk rows for int16 gather indices
    GWIN=256,      # graph window width (layer-2/pool aggregation)
    SBB=4,         # dst blocks per super-batch (PSUM residency)
    CHUNK0=2048,   # nodes per phase-0 chunk (16 tiles)
    GB2=96,        # tiles per phase-3 gather call
    USE_F32R=True,
)


def _pack_idx(ids):
    """Pack per-slot gather ids into the dma_gather SBUF layout:
    value for slot i at [i % 16, i // 16], replicated across 8 Q7 cores."""
    n = len(ids)
    assert n % 128 == 0
    arr = np.zeros((16, n // 16), np.int16)
    ar = np.arange(n)
    arr[ar % 16, ar // 16] = ids
    return np.tile(arr, (8, 1))


def preprocess(x, edge_index, batch, W1, b1, W2, b2, fc_W, fc_b, cfg):
    N, F0 = cfg["N"], cfg["F0"]
    G, NC = cfg["G"], cfg["NCORES"]
    DSTW, CH, GWIN = cfg["DSTW"], cfg["CH"], cfg["GWIN"]
    x = np.asarray(x, np.float32)
    edge_index = np.asarray(edge_index)
    batch = np.asarray(batch).astype(np.int64)
    W1 = np.asarray(W1, np.float32); b1 = np.asarray(b1, np.float32)
    W2 = np.asarray(W2, np.float32); b2 = np.asarray(b2, np.float32)
    fc_W = np.asarray(fc_W, np.float32); fc_b = np.asarray(fc_b, np.float32)
    assert x.shape == (N, F0)
    E = edge_index.shape[1]
    NPC = N // NC
    NB = -(--(-N // DSTW) // NC)      # dst blocks per core (interleaved 256-blocks)
    NCH = -(-N // CH)                 # src chunks
    NW = -(-((G + 127) // 128 * 128) // GWIN)  # graph windows (aligned)
    GPAD = ((G + 127) // 128) * 128   # padded graph count for fc blocks
    NW = max(NW, -(-GPAD // GWIN))

    loops = np.arange(N, dtype=np.int64)
    S_all = np.concatenate([edge_index[0].astype(np.int64), loops])
    D_all = np.concatenate([edge_index[1].astype(np.int64), loops])
    deg = np.bincount(D_all, minlength=N)
    dinv = (1.0 / np.sqrt(deg.astype(np.float64))).astype(np.float32)
    W_all = dinv[S_all] * dinv[D_all]
    cnt = np.bincount(batch, minlength=G)
    cntc = np.maximum(cnt, 1).astype(np.float32)
    g_of_dst = batch[D_all]

    # ---------------- P1: dst-sharded (block, chunk) tile groups ----------
    p1 = []  # per core: dict with per-(b,c) arrays
    cnt_bc = np.zeros((NC, NB, NCH), np.int64)
    gblk_d = D_all // DSTW
    for k in range(NC):
        sel = (gblk_d % NC) == k
        s1, d1, w1 = S_all[sel], D_all[sel], W_all[sel]
        b_ids = gblk_d[sel] // NC
        c_ids = s1 // CH
        order = np.lexsort((s1, c_ids, b_ids))
        s1, d1, w1, b_ids, c_ids = (a[order] for a in (s1, d1, w1, b_ids, c_ids))
        key = b_ids * NCH + c_ids
        counts = np.bincount(key, minlength=NB * NCH).reshape(NB, NCH)
        cnt_bc[k] = counts
        p1.append(dict(s=s1, d=d1, w=w1, counts=counts))
    tiles_bc = -(-np.max(cnt_bc, axis=0) // 128)          # [NB, NCH] max over cores
    # guarantee every block has >= 1 tile (should already hold via self-loops)
    empty_blocks = tiles_bc.sum(axis=1) == 0
    tiles_bc[empty_blocks, 0] = 1

    SBB = cfg["SBB"]
    sched1 = []   # list of calls: (chunk, tile_off, ntiles)
    tile_block1, tile_chunk1 = [], []
    for sb0 in range(0, NB, SBB):
        blocks = range(sb0, min(sb0 + SBB, NB))
        for c in range(NCH):
            nt = int(sum(tiles_bc[b, c] for b in blocks))
            if nt == 0:
                continue
            sched1.append((c, len(tile_block1), nt))
            for b in blocks:
                tile_block1 += [b] * int(tiles_bc[b, c])
                tile_chunk1 += [c] * int(tiles_bc[b, c])
    T1 = len(tile_block1)
    tile_block1 = np.array(tile_block1)
    first1 = np.zeros(T1, bool); last1 = np.zeros(T1, bool)
    for b in range(NB):
        w = np.where(tile_block1 == b)[0]
        first1[w[0]] = True; last1[w[-1]] = True

    # per-core P1 data arrays
    e1_idx, e1_dl, e1_nrm = [], [], []
    for k in range(NC):
        pk = p1[k]
        counts = pk["counts"]
        starts = np.zeros((NB, NCH), np.int64)
        flat = counts.ravel().cumsum()
        starts.ravel()[1:] = flat[:-1]  # row-major (b, c) offsets in sorted arrays
        starts = starts.reshape(NB, NCH)
        idx_cols = np.zeros((128, 0), np.int16)
        dl = np.full((128, T1), -1.0, np.float32)
        nrm = np.zeros((128, T1), np.float32)
        toff = 0
        for (c, toff_sched, nt) in sched1:
            ids = np.zeros(nt * 128, np.int16)
            # fill this call's tiles
            ti = toff_sched
            pos = 0
            while ti < toff_sched + nt:
                b = tile_block1[ti]
                n_tiles_grp = int(tiles_bc[b, c])
                n_edges = int(counts[b, c])
                st = int(starts[b, c])
                sl = slice(st, st + n_edges)
                loc = np.arange(n_edges)
                ids[pos * 128 + loc] = (pk["s"][sl] - c * CH).astype(np.int16)
                dl_vals = (pk["d"][sl] % DSTW).astype(np.float32)
                dl[loc % 128, ti + loc // 128] = dl_vals
                nrm[loc % 128, ti + loc // 128] = pk["w"][sl]
                ti += n_tiles_grp
                pos += n_tiles_grp
            idx_cols = np.concatenate([idx_cols, _pack_idx(ids)], axis=1)
        e1_idx.append(idx_cols); e1_dl.append(dl); e1_nrm.append(nrm)

    # ---------------- P3: src-sharded window tile groups ------------------
    p3 = []
    cnt_w = np.zeros((NC, NW), np.int64)
    gblk_s = S_all // DSTW
    for k in range(NC):
        sel = (gblk_s % NC) == k
        s2 = (gblk_s[sel] // NC) * DSTW + (S_all[sel] % DSTW)
        g2 = g_of_dst[sel]
        w2 = W_all[sel] / cntc[g2]
        wins = g2 // GWIN
        order = np.lexsort((s2, wins))
        s2, g2, w2, wins = (a[order] for a in (s2, g2, w2, wins))
        counts = np.bincount(wins, minlength=NW)
        cnt_w[k] = counts
        p3.append(dict(s=s2, g=g2, w=w2, counts=counts))
    tiles_w = -(-np.max(cnt_w, axis=0) // 128)            # [NW]
    T2 = int(tiles_w.sum())
    tile_win = np.concatenate([[w] * int(tiles_w[w]) for w in range(NW)]) if T2 else np.zeros(0, np.int64)
    first2 = np.zeros(T2, bool); last2 = np.zeros(T2, bool)
    for w in range(NW):
        ww = np.where(tile_win == w)[0]
        if len(ww):
            first2[ww[0]] = True; last2[ww[-1]] = True
    GB2 = cfg["GB2"]
    sched2 = [(t0, min(GB2, T2 - t0)) for t0 in range(0, T2, GB2)]

    e2_idx, e2_gl, e2_w2 = [], [], []
    for k in range(NC):
        pk = p3[k]
        counts = pk["counts"]
        starts = np.zeros(NW, np.int64)
        starts[1:] = counts.cumsum()[:-1]
        ids = np.zeros(T2 * 128, np.int16)
        gl = np.full((128, T2), -1.0, np.float32)
        w2a = np.zeros((128, T2), np.float32)
        for w in range(NW):
            n_edges = int(counts[w])
            if n_edges == 0:
                continue
            st = int(starts[w])
            t0 = int(np.where(tile_win == w)[0][0])
            loc = np.arange(n_edges)
            ids[t0 * 128 + loc] = pk["s"][st:st + n_edges].astype(np.int16)
            gl[loc % 128, t0 + loc // 128] = (pk["g"][st:st + n_edges] - w * GWIN).astype(np.float32)
            w2a[loc % 128, t0 + loc // 128] = pk["w"][st:st + n_edges]
        # pack idx per call
        idx_cols = np.concatenate(
            [_pack_idx(ids[t0 * 128:(t0 + nt) * 128]) for (t0, nt) in sched2], axis=1
        ) if T2 else np.zeros((128, 0), np.int16)
        e2_idx.append(idx_cols); e2_gl.append(gl); e2_w2.append(w2a)

    # ---------------- small tensors ---------------------------------------
    NGB = GPAD // 128
    fcb_eff = fc_b[None, :] + (cnt > 0)[:, None].astype(np.float32) * (b2 @ fc_W)[None, :]
    fcbm = np.zeros((128, NGB, 4), np.float32)
    for g in range(G):
        fcbm[g % 128, g // 128, :] = fcb_eff[g]
    for g in range(G, GPAD):
        fcbm[g % 128, g // 128, :] = fc_b
    iota = np.tile(np.arange(max(DSTW, GWIN), dtype=np.float32), (128, 1))

    xT = np.ascontiguousarray(x.T)

    meta = dict(cfg=cfg, NB=NB, NCH=NCH, NW=NW, NPC=NPC, T1=T1, T2=T2,
                GPAD=GPAD, NGB=NGB,
                tiles_bc=tiles_bc, sched1=sched1, tile_block1=tile_block1,
                tile_chunk1=tile_chunk1, first1=first1, last1=last1,
                tiles_w=tiles_w, tile_win=tile_win, first2=first2,
                last2=last2, sched2=sched2)

    in_maps = []
    for k in range(NC):
        in_maps.append({
            "xT": xT, "W1": W1, "b1": b1.reshape(-1, 1),
            "W2": W2, "fcW": fc_W, "fcbm": fcbm, "iota": iota,
            "e1_idx": e1_idx[k], "e1_dl": e1_dl[k], "e1_nrm": e1_nrm[k],
            "e2_idx": e2_idx[k], "e2_gl": e2_gl[k], "e2_w2": e2_w2[k],
        })
    return in_maps, meta


def build_program(meta):
    cfg = meta["cfg"]
    N, F0, F1, F2 = cfg["N"], cfg["F0"], cfg["F1"], cfg["F2"]
    NC, DSTW, CH, GWIN = cfg["NCORES"], cfg["DSTW"], cfg["CH"], cfg["GWIN"]
    CHUNK0 = cfg["CHUNK0"]
    NB, NCH, NW, NPC = meta["NB"], meta["NCH"], meta["NW"], meta["NPC"]
    T1, T2, GPAD, NGB = meta["T1"], meta["T2"], meta["GPAD"], meta["NGB"]
    SBB = cfg["SBB"]
    fr = mybir.dt.float32r if cfg["USE_F32R"] else F32
    IOTW = max(DSTW, GWIN)
    KT = F0 // 128                      # k-chunks in phase 0

    nc = bacc.Bacc("TRN2", target_bir_lowering=False, debug=False,
                   num_devices=NC)

    xT = nc.dram_tensor("xT", [F0, N], F32, kind="ExternalInput")
    W1 = nc.dram_tensor("W1", [F0, F1], F32, kind="ExternalInput")
    b1 = nc.dram_tensor("b1", [F1, 1], F32, kind="ExternalInput")
    W2 = nc.dram_tensor("W2", [F1, F2], fr, kind="ExternalInput")
    fcW = nc.dram_tensor("fcW", [F2, 4], F32, kind="ExternalInput")
    fcbm = nc.dram_tensor("fcbm", [128, NGB, 4], F32, kind="ExternalInput")
    iota = nc.dram_tensor("iota", [128, IOTW], F32, kind="ExternalInput")
    e1_idx = nc.dram_tensor("e1_idx", [128, T1 * 8], I16, kind="ExternalInput")
    e1_dl = nc.dram_tensor("e1_dl", [128, T1], F32, kind="ExternalInput")
    e1_nrm = nc.dram_tensor("e1_nrm", [128, T1], F32, kind="ExternalInput")
    e2_idx = nc.dram_tensor("e2_idx", [128, T2 * 8], I16, kind="ExternalInput")
    e2_gl = nc.dram_tensor("e2_gl", [128, T2], F32, kind="ExternalInput")
    e2_w2 = nc.dram_tensor("e2_w2", [128, T2], F32, kind="ExternalInput")

    NROW0 = -(-N // CHUNK0) * CHUNK0
    xw1d = nc.dram_tensor("xw1d", [NROW0, F1], fr)
    xw2d = nc.dram_tensor("xw2d", [NB * DSTW, F2], fr)
    ccin = nc.dram_tensor("ccin", [F2, NW * GWIN], F32)
    ccout = nc.dram_tensor("ccout", [F2, NW * GWIN], F32, addr_space="Shared")
    out = nc.dram_tensor("out", [GPAD, 4], F32, kind="ExternalOutput")

    AF = mybir.ActivationFunctionType
    AL = mybir.AluOpType

    with tile.TileContext(nc) as tc:
      with tc.tile_pool(name="const", bufs=1) as cpool:
        with (
            tc.tile_pool(name="meta1", bufs=1) as mpool,
            tc.tile_pool(name="work", bufs=2) as wpool,
            tc.tile_pool(name="psum", bufs=1, space="PSUM") as pspool,
        ):
            nc.gpsimd.load_library(library_config.mlp)

            iota_t = cpool.tile([128, IOTW], F32)
            nc.sync.dma_start(out=iota_t[:], in_=iota[:])
            W1_t = cpool.tile([128, KT, F1], F32)
            nc.sync.dma_start(out=W1_t[:], in_=W1[:].rearrange("(j p) c -> p j c", p=128))
            b1_t = cpool.tile([F1, 1], F32)
            nc.sync.dma_start(out=b1_t[:], in_=b1[:])
            W2_t = cpool.tile([F1, F2], fr)
            nc.sync.dma_start(out=W2_t[:], in_=W2[:])
            fcW_t = cpool.tile([F2, 4], F32)
            nc.sync.dma_start(out=fcW_t[:], in_=fcW[:])
            fcbm_t = cpool.tile([128, NGB, 4], F32)
            nc.sync.dma_start(out=fcbm_t[:], in_=fcbm[:])

            e1i_t = mpool.tile([128, T1 * 8], I16)
            nc.sync.dma_start(out=e1i_t[:], in_=e1_idx[:])
            e1d_t = mpool.tile([128, T1], F32)
            nc.sync.dma_start(out=e1d_t[:], in_=e1_dl[:])
            e1n_t = mpool.tile([128, T1], F32)
            nc.sync.dma_start(out=e1n_t[:], in_=e1_nrm[:])

            # ---------------- Phase 0: xw1 = x @ W1 (all nodes) ----------
            for ch in range(0, N, CHUNK0):
                cn0 = min(CHUNK0, N - ch)
                xk = []
                for j in range(KT):
                    xt = wpool.tile([128, CHUNK0], F32, tag=f"x0_{j}")
                    nc.sync.dma_start(out=xt[:, :cn0], in_=xT[128 * j:128 * (j + 1), ch:ch + cn0])
                    xk.append(xt)
                stage = wpool.tile([128, CHUNK0 // 128, F1], fr, tag="xw1s")
                for t in range(CHUNK0 // 128):
                    w = min(128, cn0 - t * 128)
                    if w <= 0:
                        break
                    ps = pspool.tile([128, F1], F32, tag="p0", bufs=2)
                    for j in range(KT):
                        nc.tensor.matmul(
                            out=ps[:w, :], lhsT=xk[j][:, t * 128:t * 128 + w],
                            rhs=W1_t[:, j, :], start=(j == 0), stop=(j == KT - 1))
                    nc.vector.tensor_copy(stage[:w, t, :], ps[:w, :])
                nc.sync.dma_start(
                    out=xw1d[ch:ch + CHUNK0, :].rearrange("(t p) f -> p t f", p=128),
                    in_=stage[:])

            # ---------------- Phase 1+2: layer-1 agg + xw2 ---------------
            tiles_bc = meta["tiles_bc"]; sched1 = meta["sched1"]
            tile_block1 = meta["tile_block1"]
            first1, last1 = meta["first1"], meta["last1"]
            agg_ps = {}
            GMAX1 = max(nt for (_, _, nt) in sched1)
            for (c, toff, nt) in sched1:
                gb = wpool.tile([128, GMAX1, F1], fr, tag="g1")
                lo = c * CH
                hi = min(lo + CH, N)
                nc.gpsimd.dma_gather(
                    out_ap=gb[:, :nt, :], in_ap=xw1d[lo:hi, :],
                    idxs_ap=e1i_t[:, toff * 8:(toff + nt) * 8],
                    num_idxs=nt * 128, num_idxs_reg=nt * 128,
                    elem_size=F1, single_packet=False)
                done_blocks = []
                for lt in range(nt):
                    gt = toff + lt
                    b = int(tile_block1[gt])
                    s = wpool.tile([128, DSTW], fr, tag="s1", bufs=8)
                    nc.vector.tensor_scalar(
                        out=s[:], in0=iota_t[:, :DSTW],
                        scalar1=e1d_t[:, gt:gt + 1], scalar2=e1n_t[:, gt:gt + 1],
                        op0=AL.is_equal, op1=AL.mult)
                    if first1[gt]:
                        agg_ps[b] = pspool.tile([F1, DSTW], F32, tag=f"agg{b % SBB}", name=f"aggps{b % SBB}")
                    nc.tensor.matmul(out=agg_ps[b][:], lhsT=gb[:, lt, :], rhs=s[:],
                                     start=bool(first1[gt]), stop=bool(last1[gt]))
                    if last1[gt]:
                        done_blocks.append(b)
                for b in done_blocks:
                    h1 = wpool.tile([F1, DSTW], fr, tag="h1")
                    nc.scalar.activation(h1[:], agg_ps[b][:], AF.Relu, bias=b1_t[:])
                    del agg_ps[b]
                    xw2s = wpool.tile([128, DSTW // 128, F2], fr, tag="xw2s")
                    for t in range(DSTW // 128):
                        ps2 = pspool.tile([128, F2], F32, tag="xw2p")
                        nc.tensor.matmul(out=ps2[:], lhsT=h1[:, t * 128:(t + 1) * 128],
                                         rhs=W2_t[:], start=True, stop=True)
                        nc.vector.tensor_copy(xw2s[:, t, :], ps2[:])
                    nc.sync.dma_start(
                        out=xw2d[b * DSTW:(b + 1) * DSTW, :].rearrange(
                            "(t p) f -> p t f", p=128),
                        in_=xw2s[:])

        # ---------------- Phase 3: layer-2 agg fused into pooling --------
        with (
            tc.tile_pool(name="meta2", bufs=1) as m2pool,
            tc.tile_pool(name="work3", bufs=2) as w3pool,
            tc.tile_pool(name="psum3", bufs=1, space="PSUM") as ps3pool,
            tc.tile_pool(name="dram", bufs=1, space="DRAM") as dpool,  # noqa: F841
        ):
            e2i_t = m2pool.tile([128, T2 * 8], I16)
            nc.sync.dma_start(out=e2i_t[:], in_=e2_idx[:])
            e2g_t = m2pool.tile([128, T2], F32)
            nc.sync.dma_start(out=e2g_t[:], in_=e2_gl[:])
            e2w_t = m2pool.tile([128, T2], F32)
            nc.sync.dma_start(out=e2w_t[:], in_=e2_w2[:])

            tile_win = meta["tile_win"]; first2, last2 = meta["first2"], meta["last2"]
            win_ps = {}
            for (t0, ntb) in meta["sched2"]:
                g2 = w3pool.tile([128, cfg["GB2"], F2], fr, tag="g2")
                nc.gpsimd.dma_gather(
                    out_ap=g2[:, :ntb, :], in_ap=xw2d[:, :],
                    idxs_ap=e2i_t[:, t0 * 8:(t0 + ntb) * 8],
                    num_idxs=ntb * 128, num_idxs_reg=ntb * 128,
                    elem_size=F2, single_packet=False)
                for lt in range(ntb):
                    gt = t0 + lt
                    w = int(tile_win[gt])
                    s2 = w3pool.tile([128, GWIN], fr, tag="s2", bufs=8)
                    nc.vector.tensor_scalar(
                        out=s2[:], in0=iota_t[:, :GWIN],
                        scalar1=e2g_t[:, gt:gt + 1], scalar2=e2w_t[:, gt:gt + 1],
                        op0=AL.is_equal, op1=AL.mult)
                    if first2[gt]:
                        win_ps[w] = ps3pool.tile([F2, GWIN], F32, tag=f"pw{w}", name=f"winps{w}")
                    nc.tensor.matmul(out=win_ps[w][:], lhsT=g2[:, lt, :], rhs=s2[:],
                                     start=bool(first2[gt]), stop=bool(last2[gt]))

            poolT = w3pool.tile([F2, NW, GWIN], F32)
            nc.vector.memset(poolT[:], 0.0)
            for w in range(NW):
                if w in win_ps:
                    nc.vector.tensor_copy(poolT[:, w, :], win_ps[w][:])
            nc.sync.dma_start(out=ccin[:], in_=poolT[:].rearrange("p a b -> p (a b)"))
            nc.gpsimd.collective_compute(
                "AllReduce", AL.add, replica_groups=[list(range(NC))],
                ins=[ccin[:]], outs=[ccout[:]])
            poolR = w3pool.tile([F2, NW * GWIN], F32)
            nc.sync.dma_start(out=poolR[:], in_=ccout[:])

            # ---------------- Phase 4: FC + log_softmax ------------------
            outsb = w3pool.tile([128, NGB, 4], F32)
            for t in range(NGB):
                ps4 = ps3pool.tile([128, 4], F32, tag="fc")
                nc.tensor.matmul(out=ps4[:], lhsT=poolR[:, t * 128:(t + 1) * 128],
                                 rhs=fcW_t[:], start=True, stop=True)
                lg = w3pool.tile([128, 4], F32, tag="lg")
                nc.vector.tensor_tensor(out=lg[:], in0=ps4[:], in1=fcbm_t[:, t, :],
                                        op=AL.add)
                mx = w3pool.tile([128, 1], F32, tag="mx")
                nc.vector.reduce_max(mx[:], lg[:], axis=mybir.AxisListType.X)
                z = w3pool.tile([128, 4], F32, tag="z")
                nc.vector.tensor_scalar(out=z[:], in0=lg[:], scalar1=mx[:],
                                        scalar2=None, op0=AL.subtract)
                ex = w3pool.tile([128, 4], F32, tag="ex")
                se = w3pool.tile([128, 1], F32, tag="se")
                nc.scalar.activation(ex[:], z[:], AF.Exp, accum_out=se[:])
                ls = w3pool.tile([128, 1], F32, tag="ls")
                nc.scalar.activation(ls[:], se[:], AF.Ln)
                nc.vector.tensor_scalar(out=outsb[:, t, :], in0=z[:], scalar1=ls[:],
                                        scalar2=None, op0=AL.subtract)
            nc.sync.dma_start(out=out[:].rearrange("(t p) c -> p t c", p=128),
                              in_=outsb[:])

    nc.compile()
    return nc


def run(inputs, cfg):
    in_maps, meta = preprocess(
        inputs["x"], inputs["edge_index"], inputs["batch"],
        inputs["W1"], inputs["b1"], inputs["W2"], inputs["b2"],
        inputs["fc_W"], inputs["fc_b"], cfg)
    nc = build_program(meta)
    res = run_bass_kernel_spmd(nc, in_maps, list(range(cfg["NCORES"])))
    return np.asarray(res.results[0]["out"][:cfg["G"]], np.float32), nc, in_maps, meta


def kernel(**inputs) -> np.ndarray:
    out, _, _, _ = run(inputs, FULL_CFG)
    return out
